# revision 25
# baseline (speedup 1.0000x reference)
"""Multi-head attention (B=2, L=2048, D=2048, H=16, d=128, RoPE, causal)
on 8 Trainium2 NeuronCores, tensor-parallel over heads (2 heads/core).

Full-bf16 matmul path (halves LDWEIGHTS, keeps the HAM clock gate at
2.4 GHz via dense PE streams). Structure per batch:
  A) QKV projections accumulate in PSUM over 16 D-chunks; RoPE reads
     PSUM once via an ACT copy (GpSimd f32 muls + DVE crossed muls/add).
  B) scores [k,q] layout with per-512-q-chunk streams; diag tiles use
     restricted q-range + zero-memset tails so AV and the denominator
     (ones-stationary matmul into [1,LQ] PSUM) keep clean full-width
     accumulation groups; causal mask is one [128,128] tri-multiply on
     GpSimd per diag tile. Normalization (reciprocal -> ones-broadcast
     matmul -> DVE mul onto pre-scaled oT) is deferred two streams so
     the slow [1,LQ] reciprocal never stalls the in-order PE queue.
  C) output projection accumulates both heads in one PSUM group over
     half-D tiles (4-deep rotation), ACT/DVE alternate the bf16 copies.
State tensors (qr/kr/v/oTs/recT) are per-512-chunk tiles: the tile
framework tracks dependencies at tile granularity, so monolithic [HD,L]
tensors would serialize phase B behind all of A and C behind the last
normalization. Host sums the 8 bf16 partials + bo in f32.
"""

import sys

sys.path.insert(0, "/opt/trn_rl_repo")

import numpy as np

B = 2
L = 2048
D = 2048
H = 16
HD = 128          # head dim
NCORES = 8
HPC = H // NCORES  # heads per core = 2
EPC = HPC * HD     # output features per core = 256
LQ = 512           # q chunk width
NCH = L // LQ      # 4 q chunks
NDC = D // 128     # 16 contraction chunks
NLT = L // 128     # 16 l-tiles
SCALE = 1.0 / np.sqrt(HD)
ROPE_BASE = 10000.0

_compiled = {}


def _rope_tables_T():
    inv_freq = 1.0 / (ROPE_BASE ** (np.arange(0, HD, 2, dtype=np.float64) / HD))
    t = np.arange(L, dtype=np.float64)
    freqs = np.outer(t, inv_freq)                    # [L, 64]
    emb = np.concatenate([freqs, freqs], axis=-1)    # [L, 128]
    cosT = np.cos(emb).T.astype(np.float32)          # [128, L]
    sinT = np.sin(emb).T.astype(np.float32)
    sinT[0:64, :] *= -1.0                            # fold rotate-half sign
    return np.ascontiguousarray(cosT), np.ascontiguousarray(sinT)


def _tri():
    # tri[k, q] = 1.0 iff k <= q  (valid region inside a diagonal 128x128)
    i = np.arange(128)
    return (i[:, None] <= i[None, :]).astype(np.float32)


def _build(with_bias):
    import concourse.bass as bass
    import concourse.tile as tile
    from concourse import bacc, mybir

    f32 = mybir.dt.float32
    bf16 = mybir.dt.bfloat16
    nc = bacc.Bacc("TRN2", target_bir_lowering=False, debug=False,
                   enable_asserts=False, num_devices=NCORES)

    xT = nc.dram_tensor("xT", [B, D, L], bf16, kind="ExternalInput").ap()
    wqT = nc.dram_tensor("wqT", [128, NDC, EPC], bf16, kind="ExternalInput").ap()
    wkT = nc.dram_tensor("wkT", [128, NDC, EPC], bf16, kind="ExternalInput").ap()
    wvT = nc.dram_tensor("wvT", [128, NDC, EPC], bf16, kind="ExternalInput").ap()
    woT = nc.dram_tensor("woT", [128, HPC, D], bf16, kind="ExternalInput").ap()
    if with_bias:
        bq = nc.dram_tensor("bq", [1, EPC], bf16, kind="ExternalInput").ap()
        bk = nc.dram_tensor("bk", [1, EPC], bf16, kind="ExternalInput").ap()
        bv = nc.dram_tensor("bv", [1, EPC], bf16, kind="ExternalInput").ap()
    cosT = nc.dram_tensor("cosT", [HD, L], f32, kind="ExternalInput").ap()
    sinT = nc.dram_tensor("sinT", [HD, L], f32, kind="ExternalInput").ap()
    tri = nc.dram_tensor("tri", [128, 128], bf16, kind="ExternalInput").ap()
    part = nc.dram_tensor("part", [B, L, D], bf16, kind="ExternalOutput").ap()

    Exp = mybir.ActivationFunctionType.Exp

    with tile.TileContext(nc) as tc:
      with tc.tile_pool(name="consts", bufs=1) as consts, \
           tc.tile_pool(name="state", bufs=1) as state, \
           tc.tile_pool(name="psr", bufs=1, space="PSUM") as psr, \
           tc.tile_pool(name="pA_x", bufs=8) as px, \
           tc.tile_pool(name="pA_scr", bufs=2) as psc, \
           tc.tile_pool(name="pA_t", bufs=2) as prt, \
           tc.tile_pool(name="pB_e", bufs=6) as pe, \
           tc.tile_pool(name="pB_rs", bufs=4) as prs, \
           tc.tile_pool(name="pC_sb", bufs=4) as pcs:
        # one shared ring of eight 1-bank PSUM tiles for all phases — pool
        # open/close barriers between phases would stall the PE for the
        # full predecessor-phase drain; per-tile WAR tracking is precise
        ps = [psr.tile([128, LQ], f32, tag=f"ps{i}", name=f"ps{i}")
              for i in range(8)]

        wq_sb = [consts.tile([128, 4, EPC], bf16, tag=f"wq{g}", name=f"wq{g}")
                 for g in range(4)]
        wk_sb = [consts.tile([128, 4, EPC], bf16, tag=f"wk{g}", name=f"wk{g}")
                 for g in range(4)]
        wv_sb = [consts.tile([128, 4, EPC], bf16, tag=f"wv{g}", name=f"wv{g}")
                 for g in range(4)]
        xt_pre = {}
        for g in range(4):
            nc.sync.dma_start(wq_sb[g][:], wqT[:, g * 4:(g + 1) * 4, :])
            nc.sync.dma_start(wk_sb[g][:], wkT[:, g * 4:(g + 1) * 4, :])
            nc.sync.dma_start(wv_sb[g][:], wvT[:, g * 4:(g + 1) * 4, :])
            if g == 0:
                for dc in range(6):
                    xt = px.tile([128, LQ], bf16, tag="xt")
                    nc.sync.dma_start(
                        xt[:], xT[0, dc * 128:(dc + 1) * 128, 0:LQ])
                    xt_pre[(0, 0, dc)] = xt
        tri_sb = consts.tile([128, 128], bf16, tag="tri")
        nc.sync.dma_start(tri_sb[:], tri)
        if with_bias:
            bq_sb = consts.tile([1, EPC], bf16, tag="bq")
            nc.sync.dma_start(bq_sb[:], bq)
            bk_sb = consts.tile([1, EPC], bf16, tag="bk")
            nc.sync.dma_start(bk_sb[:], bk)
            bv_sb = consts.tile([1, EPC], bf16, tag="bv")
            nc.sync.dma_start(bv_sb[:], bv)
            ones_row = consts.tile([1, LQ], bf16, tag="ones_row")
            nc.vector.memset(ones_row[:], 1.0)
        ones_col = consts.tile([128, 1], bf16, tag="ones_col")
        nc.vector.memset(ones_col[:], 1.0)
        ones_r128 = consts.tile([1, 128], bf16, tag="ones_r128")
        nc.vector.memset(ones_r128[:], 1.0)
        # cos/sin per chunk, DMA'd just-in-time from inside phase A;
        # wo is loaded once mid-A (needed first in phase C)
        cos_sb = [consts.tile([HD, LQ], f32, tag=f"cos{c}", name=f"cos{c}")
                  for c in range(NCH)]
        sin_sb = [consts.tile([HD, LQ], f32, tag=f"sin{c}", name=f"sin{c}")
                  for c in range(NCH)]
        wo_sb = consts.tile([128, HPC, D], bf16, tag="wo")

        def xt_get(b, c, dc):
            key = (b, c, dc)
            if key in xt_pre:
                return xt_pre.pop(key)
            xt = px.tile([128, LQ], bf16, tag="xt")
            nc.sync.dma_start(
                xt[:], xT[b, dc * 128:(dc + 1) * 128, c * LQ:(c + 1) * LQ])
            return xt

        for b in range(B):
            qr_sb = [[state.tile([HD, LQ], bf16, tag=f"qr{h}_{c}", name=f"qr{h}_{c}")
                      for c in range(NCH)] for h in range(HPC)]
            kr_sb = [[state.tile([HD, LQ], bf16, tag=f"kr{h}_{c}", name=f"kr{h}_{c}")
                      for c in range(NCH)] for h in range(HPC)]
            v_sb = [state.tile([128, 4, EPC], bf16, tag=f"v_{c}", name=f"v_{c}")
                    for c in range(NCH)]
            oTs_sb = [[state.tile([HD, LQ], bf16, tag=f"oTs{h}_{c}", name=f"oTs{h}_{c}")
                       for c in range(NCH)] for h in range(HPC)]
            recT_sb = [[state.tile([1, LQ], bf16, tag=f"recT{h}_{c}", name=f"recT{h}_{c}")
                        for c in range(NCH)] for h in range(HPC)]

            # ---------------- Phase A: QKV projections + RoPE ----------------
            # PSUM ring use: q -> ps0/ps1 (per h), k -> ps2/ps3, v -> ps4-7
            for c in range(NCH):
                q_ps = [ps[0], ps[1]]
                k_ps = [ps[2], ps[3]]
                v_ps = [ps[4 + i] for i in range(4)]
                for dc in range(NDC):
                    xt = xt_get(b, c, dc)
                    if b == 0 and dc == 4:
                        nc.sync.dma_start(
                            cos_sb[c][:], cosT[:, c * LQ:(c + 1) * LQ])
                        nc.sync.dma_start(
                            sin_sb[c][:], sinT[:, c * LQ:(c + 1) * LQ])
                        if c == 2:
                            nc.sync.dma_start(wo_sb[:], woT)
                    st = dc == 0
                    sp = (not with_bias) and dc == NDC - 1
                    for h in range(HPC):
                        nc.tensor.matmul(
                            q_ps[h][:], wq_sb[dc // 4][:, dc % 4, h * HD:(h + 1) * HD],
                            xt[:], start=st, stop=sp)
                        nc.tensor.matmul(
                            k_ps[h][:], wk_sb[dc // 4][:, dc % 4, h * HD:(h + 1) * HD],
                            xt[:], start=st, stop=sp)
                    for lt in range(4):
                        nc.tensor.matmul(
                            v_ps[lt][:, 0:EPC], xt[:, lt * 128:(lt + 1) * 128],
                            wv_sb[dc // 4][:, dc % 4, :], start=st, stop=sp)
                if with_bias:
                    for h in range(HPC):
                        nc.tensor.matmul(
                            q_ps[h][:], bq_sb[:, h * HD:(h + 1) * HD],
                            ones_row[:], start=False, stop=True)
                        nc.tensor.matmul(
                            k_ps[h][:], bk_sb[:, h * HD:(h + 1) * HD],
                            ones_row[:], start=False, stop=True)
                    for lt in range(4):
                        nc.tensor.matmul(v_ps[lt][:, 0:EPC], ones_r128[:],
                                         bv_sb[:], start=False, stop=True)

                # v copies on DVE (ACT queue backlog would gate the v banks
                # that phase B's score tiles land on)
                for lt in range(4):
                    nc.vector.tensor_copy(v_sb[c][:, lt, :], v_ps[lt][:, 0:EPC])
                # RoPE
                cs = cos_sb[c][:]
                sn = sin_sb[c][:]
                for srcs, dst in ((q_ps, qr_sb), (k_ps, kr_sb)):
                    for h in range(HPC):
                        scr = psc.tile([128, LQ], f32, tag="scr")
                        nc.scalar.copy(scr[:], srcs[h][:])
                        t1 = prt.tile([128, LQ], f32, tag="t1")
                        t2 = prt.tile([128, LQ], f32, tag="t2")
                        nc.gpsimd.tensor_mul(t1[:], scr[:], cs)
                        nc.vector.tensor_mul(t2[0:64, :], srcs[h][64:128, :],
                                             sn[0:64, :])
                        nc.vector.tensor_mul(t2[64:128, :], srcs[h][0:64, :],
                                             sn[64:128, :])
                        nc.gpsimd.tensor_add(t1[:], t1[:], t2[:])
                        nc.scalar.copy(dst[h][c][:], t1[:])

            # -------- Phase B + interleaved C: attention + output proj -------
            # ring use: sc -> ps4/ps5/ps6 rotation, out -> ps0/ps1, den ->
            # ps2/ps3 (row 0), rec -> ps7 wait; C quarter tiles -> rotation
            norm_pend = []
            sc_rot = [4, 5, 6]
            out_rot = [0, 1]
            den_rot = [2, 3]
            wo_rot = [4, 5, 6, 0]
            counters = {"sc": 0, "out": 0, "den": 0, "wo": 0}

            def nxt(name, rot):
                i = rot[counters[name] % len(rot)]
                counters[name] += 1
                return ps[i]

            def norm_flush():
                h_, c_, oT_sb_, recc_ = norm_pend.pop(0)
                rec_ps = ps[7]
                nc.tensor.matmul(rec_ps[:], ones_r128[:], recc_[:],
                                 start=True, stop=True)
                nc.vector.tensor_mul(oTs_sb[h_][c_][:], oT_sb_[:], rec_ps[:])

            def emit_c(cc):
                for lt in range(4 * cc, 4 * cc + 4):
                    for qt in range(4):
                        wo_ps = nxt("wo", wo_rot)
                        for h in range(HPC):
                            nc.tensor.matmul(
                                wo_ps[:],
                                oTs_sb[h][cc][:, (lt % 4) * 128:
                                              (lt % 4 + 1) * 128],
                                wo_sb[:, h, qt * LQ:(qt + 1) * LQ],
                                start=(h == 0), stop=(h == HPC - 1))
                        ob = pcs.tile([128, LQ], bf16, tag="ob")
                        if qt % 2 == 0:
                            nc.scalar.copy(ob[:], wo_ps[:])
                        else:
                            nc.vector.tensor_copy(ob[:], wo_ps[:])
                        nc.sync.dma_start(
                            part[b, lt * 128:(lt + 1) * 128,
                                 qt * LQ:(qt + 1) * LQ], ob[:])

            for h in range(HPC):
                for c in range(NCH):
                    ntk = 4 * (c + 1)
                    out_ps = nxt("out", out_rot)
                    den_ps = nxt("den", den_rot)

                    def flush(t, et):
                        nc.tensor.matmul(
                            out_ps[:], v_sb[t // 4][:, t % 4, h * HD:(h + 1) * HD],
                            et[:], start=(t == 0), stop=(t == ntk - 1))
                        nc.tensor.matmul(
                            den_ps[0:1, :], ones_col[:],
                            et[:], start=(t == 0), stop=(t == ntk - 1))

                    pend = []
                    for t in range(ntk):
                        s = t - 4 * c
                        qlo = s * 128 if s > 0 else 0
                        tc_, tb = t // 4, t % 4
                        sc = nxt("sc", sc_rot)
                        nc.tensor.matmul(
                            sc[:, qlo:LQ],
                            kr_sb[h][tc_][:, tb * 128:(tb + 1) * 128],
                            qr_sb[h][c][:, qlo:LQ],
                            start=True, stop=True)
                        et = pe.tile([128, LQ], bf16, tag="et")
                        if qlo:
                            nc.gpsimd.memset(et[:, 0:qlo], 0.0)
                        nc.scalar.activation(et[:, qlo:LQ], sc[:, qlo:LQ],
                                             Exp, scale=SCALE)
                        if s >= 0:
                            nc.gpsimd.tensor_mul(et[:, qlo:qlo + 128],
                                                 et[:, qlo:qlo + 128], tri_sb[:])
                        pend.append((t, et))
                        if len(pend) > 2:
                            flush(*pend.pop(0))
                    while pend:
                        flush(*pend.pop(0))

                    den_sb = prs.tile([1, LQ], f32, tag="den_sb")
                    nc.scalar.copy(den_sb[:], den_ps[0:1, :])
                    oT_sb = prs.tile([HD, LQ], bf16, tag="oT_sb")
                    nc.scalar.copy(oT_sb[:], out_ps[:])
                    recc = recT_sb[h][c]
                    with nc.allow_low_precision(reason="softmax rec in bf16"):
                        nc.vector.reciprocal(recc[:], den_sb[:])
                    norm_pend.append((h, c, oT_sb, recc))
                    if len(norm_pend) > 2:
                        norm_flush()

            if b + 1 < B:
                for dc in range(6):
                    xt = px.tile([128, LQ], bf16, tag="xt")
                    nc.sync.dma_start(
                        xt[:], xT[b + 1, dc * 128:(dc + 1) * 128, 0:LQ])
                    xt_pre[(b + 1, 0, dc)] = xt
            # C for chunks whose norms are flushed; tail norms interleave
            emit_c(0)
            emit_c(1)
            norm_flush()
            emit_c(2)
            norm_flush()
            emit_c(3)

    nc.compile()
    return nc


def _get_compiled(with_bias):
    if with_bias not in _compiled:
        _compiled[with_bias] = _build(with_bias)
    return _compiled[with_bias]


def _make_in_maps(query, Wq, bq, Wk, bk, Wv, bv, Wo, with_bias):
    import ml_dtypes
    bf = ml_dtypes.bfloat16

    xT = np.ascontiguousarray(query.transpose(0, 2, 1)).astype(bf)  # [B, D, L]
    cosT, sinT = _rope_tables_T()
    tri = _tri().astype(bf)

    def pack(wT, n):
        # [n*128, e] -> [128, n, e] so the SBUF load is per-partition contiguous
        e = wT.shape[1]
        return np.ascontiguousarray(
            wT.reshape(n, 128, e).transpose(1, 0, 2)).astype(bf)

    in_maps = []
    for c in range(NCORES):
        sl = slice(c * EPC, (c + 1) * EPC)
        m = {
            "xT": xT,
            "wqT": pack(Wq[sl].T, NDC),
            "wkT": pack(Wk[sl].T, NDC),
            "wvT": pack(Wv[sl].T, NDC),
            "woT": pack(Wo[:, sl].T, HPC),
            "cosT": cosT,
            "sinT": sinT,
            "tri": tri,
        }
        if with_bias:
            m["bq"] = np.ascontiguousarray(bq[sl][None, :]).astype(bf)
            m["bk"] = np.ascontiguousarray(bk[sl][None, :]).astype(bf)
            m["bv"] = np.ascontiguousarray(bv[sl][None, :]).astype(bf)
        in_maps.append(m)
    return in_maps


def kernel(query, Wq, bq, Wk, bk, Wv, bv, Wo, bo, _trace=False):
    from concourse.bass_utils import run_bass_kernel_spmd

    query = np.asarray(query, dtype=np.float32)
    Wq, Wk, Wv, Wo = (np.asarray(w, dtype=np.float32) for w in (Wq, Wk, Wv, Wo))
    bq_, bk_, bv_ = (np.asarray(x, dtype=np.float32) for x in (bq, bk, bv))
    bo = np.asarray(bo, dtype=np.float32)

    with_bias = bool(np.any(bq_) or np.any(bk_) or np.any(bv_))
    in_maps = _make_in_maps(query, Wq, bq_, Wk, bk_, Wv, bv_, Wo, with_bias)

    nc = _get_compiled(with_bias)
    res = run_bass_kernel_spmd(nc, in_maps, core_ids=list(range(NCORES)),
                               trace=_trace)
    out = np.zeros((B, L, D), dtype=np.float32)
    for r in res.results:
        out += r["part"].astype(np.float32)
    out += bo
    if _trace:
        kernel.last_exec_time_ns = res.exec_time_ns
        kernel.last_results = res
    return out.astype(np.float32)


# revision 26
# speedup vs baseline: 1.0107x; 1.0107x over previous
"""Multi-head attention (B=2, L=2048, D=2048, H=16, d=128, RoPE, causal)
on 8 Trainium2 NeuronCores, tensor-parallel over heads (2 heads/core).

Full-bf16 matmul path (halves LDWEIGHTS, keeps the HAM clock gate at
2.4 GHz via dense PE streams). Structure per batch:
  A) QKV projections accumulate in PSUM over 16 D-chunks; RoPE reads
     PSUM once via an ACT copy (GpSimd f32 muls + DVE crossed muls/add).
  B) scores [k,q] layout with per-512-q-chunk streams; diag tiles use
     restricted q-range + zero-memset tails so AV and the denominator
     (ones-stationary matmul into [1,LQ] PSUM) keep clean full-width
     accumulation groups; causal mask is one [128,128] tri-multiply on
     GpSimd per diag tile. Normalization (reciprocal -> ones-broadcast
     matmul -> DVE mul onto pre-scaled oT) is deferred two streams so
     the slow [1,LQ] reciprocal never stalls the in-order PE queue.
  C) output projection accumulates both heads in one PSUM group over
     half-D tiles (4-deep rotation), ACT/DVE alternate the bf16 copies.
State tensors (qr/kr/v/oTs/recT) are per-512-chunk tiles: the tile
framework tracks dependencies at tile granularity, so monolithic [HD,L]
tensors would serialize phase B behind all of A and C behind the last
normalization. Host sums the 8 bf16 partials + bo in f32.
"""

import sys

sys.path.insert(0, "/opt/trn_rl_repo")

import numpy as np

B = 2
L = 2048
D = 2048
H = 16
HD = 128          # head dim
NCORES = 8
HPC = H // NCORES  # heads per core = 2
EPC = HPC * HD     # output features per core = 256
LQ = 512           # q chunk width
NCH = L // LQ      # 4 q chunks
NDC = D // 128     # 16 contraction chunks
NLT = L // 128     # 16 l-tiles
SCALE = 1.0 / np.sqrt(HD)
ROPE_BASE = 10000.0

_compiled = {}


def _rope_tables_T():
    inv_freq = 1.0 / (ROPE_BASE ** (np.arange(0, HD, 2, dtype=np.float64) / HD))
    t = np.arange(L, dtype=np.float64)
    freqs = np.outer(t, inv_freq)                    # [L, 64]
    emb = np.concatenate([freqs, freqs], axis=-1)    # [L, 128]
    cosT = np.cos(emb).T.astype(np.float32)          # [128, L]
    sinT = np.sin(emb).T.astype(np.float32)
    sinT[0:64, :] *= -1.0                            # fold rotate-half sign
    return np.ascontiguousarray(cosT), np.ascontiguousarray(sinT)


def _tri():
    # tri[k, q] = 1.0 iff k <= q  (valid region inside a diagonal 128x128)
    i = np.arange(128)
    return (i[:, None] <= i[None, :]).astype(np.float32)


def _build(with_bias):
    import concourse.bass as bass
    import concourse.tile as tile
    from concourse import bacc, mybir

    f32 = mybir.dt.float32
    bf16 = mybir.dt.bfloat16
    nc = bacc.Bacc("TRN2", target_bir_lowering=False, debug=False,
                   enable_asserts=False, num_devices=NCORES)

    xT = nc.dram_tensor("xT", [B, D, L], bf16, kind="ExternalInput").ap()
    wqT = nc.dram_tensor("wqT", [128, NDC, EPC], bf16, kind="ExternalInput").ap()
    wkT = nc.dram_tensor("wkT", [128, NDC, EPC], bf16, kind="ExternalInput").ap()
    wvT = nc.dram_tensor("wvT", [128, NDC, EPC], bf16, kind="ExternalInput").ap()
    woT = nc.dram_tensor("woT", [128, HPC, D], bf16, kind="ExternalInput").ap()
    if with_bias:
        bq = nc.dram_tensor("bq", [1, EPC], bf16, kind="ExternalInput").ap()
        bk = nc.dram_tensor("bk", [1, EPC], bf16, kind="ExternalInput").ap()
        bv = nc.dram_tensor("bv", [1, EPC], bf16, kind="ExternalInput").ap()
    cosT = nc.dram_tensor("cosT", [HD, L], f32, kind="ExternalInput").ap()
    sinT = nc.dram_tensor("sinT", [HD, L], f32, kind="ExternalInput").ap()
    tri = nc.dram_tensor("tri", [128, 128], bf16, kind="ExternalInput").ap()
    part = nc.dram_tensor("part", [B, L, D], bf16, kind="ExternalOutput").ap()

    Exp = mybir.ActivationFunctionType.Exp

    with tile.TileContext(nc) as tc:
      with tc.tile_pool(name="consts", bufs=1) as consts, \
           tc.tile_pool(name="state", bufs=1) as state, \
           tc.tile_pool(name="psr", bufs=1, space="PSUM") as psr, \
           tc.tile_pool(name="pA_x", bufs=8) as px, \
           tc.tile_pool(name="pA_scr", bufs=2) as psc, \
           tc.tile_pool(name="pA_t", bufs=2) as prt, \
           tc.tile_pool(name="pB_e", bufs=6) as pe, \
           tc.tile_pool(name="pB_rs", bufs=4) as prs, \
           tc.tile_pool(name="pC_sb", bufs=4) as pcs:
        # one shared ring of eight 1-bank PSUM tiles for all phases — pool
        # open/close barriers between phases would stall the PE for the
        # full predecessor-phase drain; per-tile WAR tracking is precise
        ps = [psr.tile([128, LQ], f32, tag=f"ps{i}", name=f"ps{i}")
              for i in range(8)]

        wq_sb = [consts.tile([128, 4, EPC], bf16, tag=f"wq{g}", name=f"wq{g}")
                 for g in range(4)]
        wk_sb = [consts.tile([128, 4, EPC], bf16, tag=f"wk{g}", name=f"wk{g}")
                 for g in range(4)]
        wv_sb = [consts.tile([128, 4, EPC], bf16, tag=f"wv{g}", name=f"wv{g}")
                 for g in range(4)]
        xt_pre = {}
        for g in range(4):
            nc.sync.dma_start(wq_sb[g][:], wqT[:, g * 4:(g + 1) * 4, :])
            nc.sync.dma_start(wk_sb[g][:], wkT[:, g * 4:(g + 1) * 4, :])
            nc.sync.dma_start(wv_sb[g][:], wvT[:, g * 4:(g + 1) * 4, :])
            if g == 0:
                for dc in range(6):
                    xt = px.tile([128, LQ], bf16, tag="xt")
                    nc.sync.dma_start(
                        xt[:], xT[0, dc * 128:(dc + 1) * 128, 0:LQ])
                    xt_pre[(0, 0, dc)] = xt
        tri_sb = consts.tile([128, 128], bf16, tag="tri")
        nc.sync.dma_start(tri_sb[:], tri)
        if with_bias:
            bq_sb = consts.tile([1, EPC], bf16, tag="bq")
            nc.sync.dma_start(bq_sb[:], bq)
            bk_sb = consts.tile([1, EPC], bf16, tag="bk")
            nc.sync.dma_start(bk_sb[:], bk)
            bv_sb = consts.tile([1, EPC], bf16, tag="bv")
            nc.sync.dma_start(bv_sb[:], bv)
            ones_row = consts.tile([1, LQ], bf16, tag="ones_row")
            nc.vector.memset(ones_row[:], 1.0)
        ones_col = consts.tile([128, 1], bf16, tag="ones_col")
        nc.vector.memset(ones_col[:], 1.0)
        ones_r128 = consts.tile([1, 128], bf16, tag="ones_r128")
        nc.vector.memset(ones_r128[:], 1.0)
        # cos/sin per chunk, DMA'd just-in-time from inside phase A;
        # wo is loaded once mid-A (needed first in phase C)
        cos_sb = [consts.tile([HD, LQ], f32, tag=f"cos{c}", name=f"cos{c}")
                  for c in range(NCH)]
        sin_sb = [consts.tile([HD, LQ], f32, tag=f"sin{c}", name=f"sin{c}")
                  for c in range(NCH)]
        wo_sb = consts.tile([128, HPC, D], bf16, tag="wo")

        def xt_get(b, c, dc):
            key = (b, c, dc)
            if key in xt_pre:
                return xt_pre.pop(key)
            xt = px.tile([128, LQ], bf16, tag="xt")
            nc.sync.dma_start(
                xt[:], xT[b, dc * 128:(dc + 1) * 128, c * LQ:(c + 1) * LQ])
            return xt

        for b in range(B):
            qr_sb = [[state.tile([HD, LQ], bf16, tag=f"qr{h}_{c}", name=f"qr{h}_{c}")
                      for c in range(NCH)] for h in range(HPC)]
            kr_sb = [[state.tile([HD, LQ], bf16, tag=f"kr{h}_{c}", name=f"kr{h}_{c}")
                      for c in range(NCH)] for h in range(HPC)]
            v_sb = [state.tile([128, 4, EPC], bf16, tag=f"v_{c}", name=f"v_{c}")
                    for c in range(NCH)]
            oTs_sb = [[state.tile([HD, LQ], bf16, tag=f"oTs{h}_{c}", name=f"oTs{h}_{c}")
                       for c in range(NCH)] for h in range(HPC)]
            recT_sb = [[state.tile([1, LQ], bf16, tag=f"recT{h}_{c}", name=f"recT{h}_{c}")
                        for c in range(NCH)] for h in range(HPC)]

            # ---------------- Phase A: QKV projections + RoPE ----------------
            # PSUM ring use: q -> ps0/ps1 (per h), k -> ps2/ps3, v -> ps4-7
            for c in range(NCH):
                q_ps = [ps[0], ps[1]]
                k_ps = [ps[2], ps[3]]
                v_ps = [ps[4 + i] for i in range(4)]
                for dc in range(NDC):
                    xt = xt_get(b, c, dc)
                    if b == 0 and dc == 4:
                        nc.sync.dma_start(
                            cos_sb[c][:], cosT[:, c * LQ:(c + 1) * LQ])
                        nc.sync.dma_start(
                            sin_sb[c][:], sinT[:, c * LQ:(c + 1) * LQ])
                        if c == 2:
                            nc.sync.dma_start(wo_sb[:], woT)
                    st = dc == 0
                    sp = (not with_bias) and dc == NDC - 1
                    for h in range(HPC):
                        nc.tensor.matmul(
                            q_ps[h][:], wq_sb[dc // 4][:, dc % 4, h * HD:(h + 1) * HD],
                            xt[:], start=st, stop=sp)
                        nc.tensor.matmul(
                            k_ps[h][:], wk_sb[dc // 4][:, dc % 4, h * HD:(h + 1) * HD],
                            xt[:], start=st, stop=sp)
                    for lt in range(4):
                        nc.tensor.matmul(
                            v_ps[lt][:, 0:EPC], xt[:, lt * 128:(lt + 1) * 128],
                            wv_sb[dc // 4][:, dc % 4, :], start=st, stop=sp)
                if with_bias:
                    for h in range(HPC):
                        nc.tensor.matmul(
                            q_ps[h][:], bq_sb[:, h * HD:(h + 1) * HD],
                            ones_row[:], start=False, stop=True)
                        nc.tensor.matmul(
                            k_ps[h][:], bk_sb[:, h * HD:(h + 1) * HD],
                            ones_row[:], start=False, stop=True)
                    for lt in range(4):
                        nc.tensor.matmul(v_ps[lt][:, 0:EPC], ones_r128[:],
                                         bv_sb[:], start=False, stop=True)

                for lt in range(4):
                    nc.scalar.copy(v_sb[c][:, lt, :], v_ps[lt][:, 0:EPC])
                # RoPE
                cs = cos_sb[c][:]
                sn = sin_sb[c][:]
                for srcs, dst in ((q_ps, qr_sb), (k_ps, kr_sb)):
                    for h in range(HPC):
                        scr = psc.tile([128, LQ], f32, tag="scr")
                        nc.scalar.copy(scr[:], srcs[h][:])
                        t1 = prt.tile([128, LQ], f32, tag="t1")
                        t2 = prt.tile([128, LQ], f32, tag="t2")
                        nc.gpsimd.tensor_mul(t1[:], scr[:], cs)
                        nc.vector.tensor_mul(t2[0:64, :], srcs[h][64:128, :],
                                             sn[0:64, :])
                        nc.vector.tensor_mul(t2[64:128, :], srcs[h][0:64, :],
                                             sn[64:128, :])
                        nc.gpsimd.tensor_add(t1[:], t1[:], t2[:])
                        nc.scalar.copy(dst[h][c][:], t1[:])

            # -------- Phase B + interleaved C: attention + output proj -------
            # ring use: sc -> ps4/ps5/ps6 rotation, out -> ps0/ps1, den ->
            # ps2/ps3 (row 0), rec -> ps7 wait; C quarter tiles -> rotation
            norm_pend = []
            sc_rot = [4, 5, 6]
            out_rot = [0, 1]
            den_rot = [2, 3]
            wo_rot = [4, 5, 6, 0]
            counters = {"sc": 0, "out": 0, "den": 0, "wo": 0}

            def nxt(name, rot):
                i = rot[counters[name] % len(rot)]
                counters[name] += 1
                return ps[i]

            def norm_flush():
                h_, c_, oT_sb_, recc_ = norm_pend.pop(0)
                rec_ps = ps[7]
                nc.tensor.matmul(rec_ps[:], ones_r128[:], recc_[:],
                                 start=True, stop=True)
                nc.vector.tensor_mul(oTs_sb[h_][c_][:], oT_sb_[:], rec_ps[:])

            def emit_c(cc):
                for lt in range(4 * cc, 4 * cc + 4):
                    for qt in range(4):
                        wo_ps = nxt("wo", wo_rot)
                        for h in range(HPC):
                            nc.tensor.matmul(
                                wo_ps[:],
                                oTs_sb[h][cc][:, (lt % 4) * 128:
                                              (lt % 4 + 1) * 128],
                                wo_sb[:, h, qt * LQ:(qt + 1) * LQ],
                                start=(h == 0), stop=(h == HPC - 1))
                        ob = pcs.tile([128, LQ], bf16, tag="ob")
                        if qt % 2 == 0:
                            nc.scalar.copy(ob[:], wo_ps[:])
                        else:
                            nc.vector.tensor_copy(ob[:], wo_ps[:])
                        nc.sync.dma_start(
                            part[b, lt * 128:(lt + 1) * 128,
                                 qt * LQ:(qt + 1) * LQ], ob[:])

            for h in range(HPC):
                for c in range(NCH):
                    ntk = 4 * (c + 1)
                    out_ps = nxt("out", out_rot)
                    den_ps = nxt("den", den_rot)

                    def flush(t, et):
                        nc.tensor.matmul(
                            out_ps[:], v_sb[t // 4][:, t % 4, h * HD:(h + 1) * HD],
                            et[:], start=(t == 0), stop=(t == ntk - 1))
                        nc.tensor.matmul(
                            den_ps[0:1, :], ones_col[:],
                            et[:], start=(t == 0), stop=(t == ntk - 1))

                    pend = []
                    for t in range(ntk):
                        s = t - 4 * c
                        qlo = s * 128 if s > 0 else 0
                        tc_, tb = t // 4, t % 4
                        sc = nxt("sc", sc_rot)
                        nc.tensor.matmul(
                            sc[:, qlo:LQ],
                            kr_sb[h][tc_][:, tb * 128:(tb + 1) * 128],
                            qr_sb[h][c][:, qlo:LQ],
                            start=True, stop=True)
                        et = pe.tile([128, LQ], bf16, tag="et")
                        if qlo:
                            nc.gpsimd.memset(et[:, 0:qlo], 0.0)
                        nc.scalar.activation(et[:, qlo:LQ], sc[:, qlo:LQ],
                                             Exp, scale=SCALE)
                        if s >= 0:
                            nc.gpsimd.tensor_mul(et[:, qlo:qlo + 128],
                                                 et[:, qlo:qlo + 128], tri_sb[:])
                        pend.append((t, et))
                        if len(pend) > 2:
                            flush(*pend.pop(0))
                    while pend:
                        flush(*pend.pop(0))

                    den_sb = prs.tile([1, LQ], f32, tag="den_sb")
                    nc.scalar.copy(den_sb[:], den_ps[0:1, :])
                    oT_sb = prs.tile([HD, LQ], bf16, tag="oT_sb")
                    nc.scalar.copy(oT_sb[:], out_ps[:])
                    recc = recT_sb[h][c]
                    with nc.allow_low_precision(reason="softmax rec in bf16"):
                        nc.vector.reciprocal(recc[:], den_sb[:])
                    norm_pend.append((h, c, oT_sb, recc))
                    if len(norm_pend) > 2:
                        norm_flush()

            if b + 1 < B:
                for dc in range(6):
                    xt = px.tile([128, LQ], bf16, tag="xt")
                    nc.sync.dma_start(
                        xt[:], xT[b + 1, dc * 128:(dc + 1) * 128, 0:LQ])
                    xt_pre[(b + 1, 0, dc)] = xt
            # C for chunks whose norms are flushed; tail norms interleave
            emit_c(0)
            emit_c(1)
            norm_flush()
            emit_c(2)
            norm_flush()
            emit_c(3)

    nc.compile()
    return nc


def _get_compiled(with_bias):
    if with_bias not in _compiled:
        _compiled[with_bias] = _build(with_bias)
    return _compiled[with_bias]


def _make_in_maps(query, Wq, bq, Wk, bk, Wv, bv, Wo, with_bias):
    import ml_dtypes
    bf = ml_dtypes.bfloat16

    xT = np.ascontiguousarray(query.transpose(0, 2, 1)).astype(bf)  # [B, D, L]
    cosT, sinT = _rope_tables_T()
    tri = _tri().astype(bf)

    def pack(wT, n):
        # [n*128, e] -> [128, n, e] so the SBUF load is per-partition contiguous
        e = wT.shape[1]
        return np.ascontiguousarray(
            wT.reshape(n, 128, e).transpose(1, 0, 2)).astype(bf)

    in_maps = []
    for c in range(NCORES):
        sl = slice(c * EPC, (c + 1) * EPC)
        m = {
            "xT": xT,
            "wqT": pack(Wq[sl].T, NDC),
            "wkT": pack(Wk[sl].T, NDC),
            "wvT": pack(Wv[sl].T, NDC),
            "woT": pack(Wo[:, sl].T, HPC),
            "cosT": cosT,
            "sinT": sinT,
            "tri": tri,
        }
        if with_bias:
            m["bq"] = np.ascontiguousarray(bq[sl][None, :]).astype(bf)
            m["bk"] = np.ascontiguousarray(bk[sl][None, :]).astype(bf)
            m["bv"] = np.ascontiguousarray(bv[sl][None, :]).astype(bf)
        in_maps.append(m)
    return in_maps


def kernel(query, Wq, bq, Wk, bk, Wv, bv, Wo, bo, _trace=False):
    from concourse.bass_utils import run_bass_kernel_spmd

    query = np.asarray(query, dtype=np.float32)
    Wq, Wk, Wv, Wo = (np.asarray(w, dtype=np.float32) for w in (Wq, Wk, Wv, Wo))
    bq_, bk_, bv_ = (np.asarray(x, dtype=np.float32) for x in (bq, bk, bv))
    bo = np.asarray(bo, dtype=np.float32)

    with_bias = bool(np.any(bq_) or np.any(bk_) or np.any(bv_))
    in_maps = _make_in_maps(query, Wq, bq_, Wk, bk_, Wv, bv_, Wo, with_bias)

    nc = _get_compiled(with_bias)
    res = run_bass_kernel_spmd(nc, in_maps, core_ids=list(range(NCORES)),
                               trace=_trace)
    out = np.zeros((B, L, D), dtype=np.float32)
    for r in res.results:
        out += r["part"].astype(np.float32)
    out += bo
    if _trace:
        kernel.last_exec_time_ns = res.exec_time_ns
        kernel.last_results = res
    return out.astype(np.float32)


# revision 27
# speedup vs baseline: 1.0252x; 1.0143x over previous
"""Multi-head attention (B=2, L=2048, D=2048, H=16, d=128, RoPE, causal)
on 8 Trainium2 NeuronCores, tensor-parallel over heads (2 heads/core).

Full-bf16 matmul path (halves LDWEIGHTS, keeps the HAM clock gate at
2.4 GHz via dense PE streams). Structure per batch:
  A) QKV projections accumulate in PSUM over 16 D-chunks; RoPE reads
     PSUM once via an ACT copy (GpSimd f32 muls + DVE crossed muls/add).
  B) scores [k,q] layout with per-512-q-chunk streams; diag tiles use
     restricted q-range + zero-memset tails so AV and the denominator
     (ones-stationary matmul into [1,LQ] PSUM) keep clean full-width
     accumulation groups; causal mask is one [128,128] tri-multiply on
     GpSimd per diag tile. Normalization (reciprocal -> ones-broadcast
     matmul -> DVE mul onto pre-scaled oT) is deferred two streams so
     the slow [1,LQ] reciprocal never stalls the in-order PE queue.
  C) output projection accumulates both heads in one PSUM group over
     half-D tiles (4-deep rotation), ACT/DVE alternate the bf16 copies.
State tensors (qr/kr/v/oTs/recT) are per-512-chunk tiles: the tile
framework tracks dependencies at tile granularity, so monolithic [HD,L]
tensors would serialize phase B behind all of A and C behind the last
normalization. Host sums the 8 bf16 partials + bo in f32.
"""

import sys

sys.path.insert(0, "/opt/trn_rl_repo")

import numpy as np

B = 2
L = 2048
D = 2048
H = 16
HD = 128          # head dim
NCORES = 8
HPC = H // NCORES  # heads per core = 2
EPC = HPC * HD     # output features per core = 256
LQ = 512           # q chunk width
NCH = L // LQ      # 4 q chunks
NDC = D // 128     # 16 contraction chunks
NLT = L // 128     # 16 l-tiles
SCALE = 1.0 / np.sqrt(HD)
ROPE_BASE = 10000.0

_compiled = {}


def _rope_tables_T():
    inv_freq = 1.0 / (ROPE_BASE ** (np.arange(0, HD, 2, dtype=np.float64) / HD))
    t = np.arange(L, dtype=np.float64)
    freqs = np.outer(t, inv_freq)                    # [L, 64]
    emb = np.concatenate([freqs, freqs], axis=-1)    # [L, 128]
    cosT = np.cos(emb).T.astype(np.float32)          # [128, L]
    sinT = np.sin(emb).T.astype(np.float32)
    sinT[0:64, :] *= -1.0                            # fold rotate-half sign
    return np.ascontiguousarray(cosT), np.ascontiguousarray(sinT)


def _tri():
    # tri[k, q] = 1.0 iff k <= q  (valid region inside a diagonal 128x128)
    i = np.arange(128)
    return (i[:, None] <= i[None, :]).astype(np.float32)


def _build(with_bias):
    import concourse.bass as bass
    import concourse.tile as tile
    from concourse import bacc, mybir

    f32 = mybir.dt.float32
    bf16 = mybir.dt.bfloat16
    nc = bacc.Bacc("TRN2", target_bir_lowering=False, debug=False,
                   enable_asserts=False, num_devices=NCORES)

    xT = nc.dram_tensor("xT", [B, D, L], bf16, kind="ExternalInput").ap()
    wqT = nc.dram_tensor("wqT", [128, NDC, EPC], bf16, kind="ExternalInput").ap()
    wkT = nc.dram_tensor("wkT", [128, NDC, EPC], bf16, kind="ExternalInput").ap()
    wvT = nc.dram_tensor("wvT", [128, NDC, EPC], bf16, kind="ExternalInput").ap()
    woT = nc.dram_tensor("woT", [128, HPC, D], bf16, kind="ExternalInput").ap()
    if with_bias:
        bq = nc.dram_tensor("bq", [1, EPC], bf16, kind="ExternalInput").ap()
        bk = nc.dram_tensor("bk", [1, EPC], bf16, kind="ExternalInput").ap()
        bv = nc.dram_tensor("bv", [1, EPC], bf16, kind="ExternalInput").ap()
    cosT = nc.dram_tensor("cosT", [HD, L], f32, kind="ExternalInput").ap()
    sinT = nc.dram_tensor("sinT", [HD, L], f32, kind="ExternalInput").ap()
    tri = nc.dram_tensor("tri", [128, 128], bf16, kind="ExternalInput").ap()
    part = nc.dram_tensor("part", [B, L, D], bf16, kind="ExternalOutput").ap()

    Exp = mybir.ActivationFunctionType.Exp

    with tile.TileContext(nc) as tc:
      with tc.tile_pool(name="consts", bufs=1) as consts, \
           tc.tile_pool(name="state", bufs=1) as state, \
           tc.tile_pool(name="pA_x", bufs=8) as px:
        # first x tiles for (b0,c0) go first, then weight chunks in dc
        # order so the dc-loop's dependencies land just-in-time
        wq_sb = [consts.tile([128, 4, EPC], bf16, tag=f"wq{g}", name=f"wq{g}")
                 for g in range(4)]
        wk_sb = [consts.tile([128, 4, EPC], bf16, tag=f"wk{g}", name=f"wk{g}")
                 for g in range(4)]
        wv_sb = [consts.tile([128, 4, EPC], bf16, tag=f"wv{g}", name=f"wv{g}")
                 for g in range(4)]
        # defined below; primed here so the g=0 weight chunks and the
        # first x tiles beat the remaining 4.5MB of weights onto the queues
        xt_pre = {}
        tri_sb = consts.tile([128, 128], bf16, tag="tri")
        for g in range(4):
            nc.sync.dma_start(wq_sb[g][:], wqT[:, g * 4:(g + 1) * 4, :])
            nc.sync.dma_start(wk_sb[g][:], wkT[:, g * 4:(g + 1) * 4, :])
            nc.sync.dma_start(wv_sb[g][:], wvT[:, g * 4:(g + 1) * 4, :])
            if g == 0:
                for dc in range(6):
                    xt = px.tile([128, LQ], bf16, tag="xt")
                    nc.sync.dma_start(
                        xt[:], xT[0, dc * 128:(dc + 1) * 128, 0:LQ])
                    xt_pre[(0, 0, dc)] = xt
        nc.sync.dma_start(tri_sb[:], tri)
        if with_bias:
            bq_sb = consts.tile([1, EPC], bf16, tag="bq")
            nc.sync.dma_start(bq_sb[:], bq)
            bk_sb = consts.tile([1, EPC], bf16, tag="bk")
            nc.sync.dma_start(bk_sb[:], bk)
            bv_sb = consts.tile([1, EPC], bf16, tag="bv")
            nc.sync.dma_start(bv_sb[:], bv)
            ones_row = consts.tile([1, LQ], bf16, tag="ones_row")
            nc.vector.memset(ones_row[:], 1.0)
        ones_col = consts.tile([128, 1], bf16, tag="ones_col")
        nc.vector.memset(ones_col[:], 1.0)
        ones_r128 = consts.tile([1, 128], bf16, tag="ones_r128")
        nc.vector.memset(ones_r128[:], 1.0)
        # cos/sin/wo are not needed until RoPE/phase C; loaded below after
        # the first x tiles are in flight
        cos_sb = consts.tile([HD, L], f32, tag="cos")
        sin_sb = consts.tile([HD, L], f32, tag="sin")
        wo_sb = consts.tile([128, HPC, D], bf16, tag="wo")

        # cross-batch xt prefetch: tiles whose DMA was already issued

        def xt_get(b, c, dc):
            key = (b, c, dc)
            if key in xt_pre:
                return xt_pre.pop(key)
            xt = px.tile([128, LQ], bf16, tag="xt")
            nc.sync.dma_start(
                xt[:], xT[b, dc * 128:(dc + 1) * 128, c * LQ:(c + 1) * LQ])
            return xt

        for b in range(B):
            # per-batch, per-chunk SBUF state (chunked to keep the tile
            # framework's dependency tracking fine-grained)
            qr_sb = [[state.tile([HD, LQ], bf16, tag=f"qr{h}_{c}", name=f"qr{h}_{c}")
                      for c in range(NCH)] for h in range(HPC)]
            kr_sb = [[state.tile([HD, LQ], bf16, tag=f"kr{h}_{c}", name=f"kr{h}_{c}")
                      for c in range(NCH)] for h in range(HPC)]
            v_sb = [state.tile([128, 4, EPC], bf16, tag=f"v_{c}", name=f"v_{c}")
                    for c in range(NCH)]
            oTs_sb = [[state.tile([HD, LQ], bf16, tag=f"oTs{h}_{c}", name=f"oTs{h}_{c}")
                       for c in range(NCH)] for h in range(HPC)]
            recT_sb = [[state.tile([1, LQ], bf16, tag=f"recT{h}_{c}", name=f"recT{h}_{c}")
                        for c in range(NCH)] for h in range(HPC)]

            # ---------------- Phase A: QKV projections + RoPE ----------------
            with tc.tile_pool(name="pA_ps", bufs=1, space="PSUM") as pA, \
                 tc.tile_pool(name="pA_scr", bufs=2) as psc, \
                 tc.tile_pool(name="pA_t", bufs=2) as prt:
             for c in range(NCH):
                if True:
                    q_ps = pA.tile([128, HPC, LQ], f32, tag="q_ps")
                    k_ps = pA.tile([128, HPC, LQ], f32, tag="k_ps")
                    v_ps = [pA.tile([128, EPC], f32, tag=f"v_ps{i}",
                                    name=f"v_ps{i}") for i in range(4)]
                    for dc in range(NDC):
                        xt = xt_get(b, c, dc)
                        if b == 0 and c == 0 and dc == 0:
                            # inject deferred const loads behind the first
                            # x tile so phase A starts as early as possible
                            nc.sync.dma_start(cos_sb[:], cosT)
                            nc.sync.dma_start(sin_sb[:], sinT)
                            nc.sync.dma_start(wo_sb[:], woT)
                        st = dc == 0
                        sp = (not with_bias) and dc == NDC - 1
                        for h in range(HPC):
                            nc.tensor.matmul(
                                q_ps[:, h], wq_sb[dc // 4][:, dc % 4, h * HD:(h + 1) * HD],
                                xt[:], start=st, stop=sp)
                            nc.tensor.matmul(
                                k_ps[:, h], wk_sb[dc // 4][:, dc % 4, h * HD:(h + 1) * HD],
                                xt[:], start=st, stop=sp)
                        for lt in range(4):
                            nc.tensor.matmul(
                                v_ps[lt][:], xt[:, lt * 128:(lt + 1) * 128],
                                wv_sb[dc // 4][:, dc % 4, :], start=st, stop=sp)
                    if with_bias:
                        for h in range(HPC):
                            nc.tensor.matmul(
                                q_ps[:, h], bq_sb[:, h * HD:(h + 1) * HD],
                                ones_row[:], start=False, stop=True)
                            nc.tensor.matmul(
                                k_ps[:, h], bk_sb[:, h * HD:(h + 1) * HD],
                                ones_row[:], start=False, stop=True)
                        for lt in range(4):
                            nc.tensor.matmul(v_ps[lt][:], ones_r128[:],
                                             bv_sb[:], start=False, stop=True)

                    # free the v banks first: phase B's first PSUM tiles
                    # land on them
                    for lt in range(4):
                        nc.scalar.copy(v_sb[c][:, lt, :], v_ps[lt][:])
                    # RoPE: one ACT copy out of PSUM, aligned muls on GpSimd,
                    # crossed muls straight from PSUM on DVE, add+cast split
                    cs = cos_sb[:, c * LQ:(c + 1) * LQ]
                    sn = sin_sb[:, c * LQ:(c + 1) * LQ]
                    for src, dst in ((q_ps, qr_sb), (k_ps, kr_sb)):
                        scr = psc.tile([128, HPC, LQ], f32, tag="scr")
                        nc.scalar.copy(scr[:], src[:])
                        for h in range(HPC):
                            t1 = prt.tile([128, LQ], f32, tag="t1")
                            t2 = prt.tile([128, LQ], f32, tag="t2")
                            nc.gpsimd.tensor_mul(t1[:], scr[:, h], cs)
                            nc.vector.tensor_mul(t2[0:64, :], src[64:128, h],
                                                 sn[0:64, :])
                            nc.vector.tensor_mul(t2[64:128, :], src[0:64, h],
                                                 sn[64:128, :])
                            # add in f32 on Pool, cast on ACT: a dtype-
                            # converting tensor_tensor on DVE runs ~3x slower
                            nc.gpsimd.tensor_add(t1[:], t1[:], t2[:])
                            nc.scalar.copy(dst[h][c][:], t1[:])

            # ---------------- Phase B: attention ----------------
            with tc.tile_pool(name="pB_sc", bufs=3, space="PSUM") as pb, \
                 tc.tile_pool(name="pB_o", bufs=2, space="PSUM") as po, \
                 tc.tile_pool(name="pB_d", bufs=2, space="PSUM") as pd, \
                 tc.tile_pool(name="pB_r", bufs=1, space="PSUM") as pr, \
                 tc.tile_pool(name="pB_e", bufs=6) as pe, \
                 tc.tile_pool(name="pB_rs", bufs=4) as prs:
             norm_pend = []

             def norm_flush():
                # two-stream-deferred: the reciprocal is long done, so the
                # broadcast matmul doesn't stall the in-order PE queue
                h_, c_, oT_sb_, recc_ = norm_pend.pop(0)
                rec_ps = pr.tile([128, LQ], f32, tag="rec_ps")
                nc.tensor.matmul(rec_ps[:], ones_r128[:], recc_[:],
                                 start=True, stop=True)
                nc.vector.tensor_mul(oTs_sb[h_][c_][:], oT_sb_[:], rec_ps[:])

             for h in range(HPC):
                for c in range(NCH):
                    ntk = 4 * (c + 1)
                    out_ps = po.tile([HD, LQ], f32, tag="out_ps")
                    den_ps = pd.tile([1, LQ], f32, tag="den_ps")

                    def flush(t, et):
                        nc.tensor.matmul(
                            out_ps[:], v_sb[t // 4][:, t % 4, h * HD:(h + 1) * HD],
                            et[:], start=(t == 0), stop=(t == ntk - 1))
                        nc.tensor.matmul(
                            den_ps[:], ones_col[:],
                            et[:], start=(t == 0), stop=(t == ntk - 1))

                    pend = []
                    for t in range(ntk):
                        s = t - 4 * c
                        qlo = s * 128 if s > 0 else 0
                        tc_, tb = t // 4, t % 4
                        sc = pb.tile([128, LQ], f32, tag="sc")
                        nc.tensor.matmul(
                            sc[:, qlo:LQ],
                            kr_sb[h][tc_][:, tb * 128:(tb + 1) * 128],
                            qr_sb[h][c][:, qlo:LQ],
                            start=True, stop=True)
                        et = pe.tile([128, LQ], bf16, tag="et")
                        if qlo:
                            nc.gpsimd.memset(et[:, 0:qlo], 0.0)
                        nc.scalar.activation(et[:, qlo:LQ], sc[:, qlo:LQ],
                                             Exp, scale=SCALE)
                        if s >= 0:
                            nc.gpsimd.tensor_mul(et[:, qlo:qlo + 128],
                                                 et[:, qlo:qlo + 128], tri_sb[:])
                        pend.append((t, et))
                        if len(pend) > 2:
                            flush(*pend.pop(0))
                    while pend:
                        flush(*pend.pop(0))

                    # drain PSUM via fast ACT copies; reciprocal runs SBUF-side
                    den_sb = prs.tile([1, LQ], f32, tag="den_sb")
                    nc.scalar.copy(den_sb[:], den_ps[:])
                    oT_sb = prs.tile([HD, LQ], bf16, tag="oT_sb")
                    nc.scalar.copy(oT_sb[:], out_ps[:])
                    recc = recT_sb[h][c]
                    with nc.allow_low_precision(reason="softmax rec in bf16"):
                        nc.vector.reciprocal(recc[:], den_sb[:])
                    norm_pend.append((h, c, oT_sb, recc))
                    if len(norm_pend) > 2:
                        norm_flush()
             while norm_pend:
                norm_flush()

            # ---------------- Phase C: output projection ----------------
            with tc.tile_pool(name="pC_ps", bufs=4, space="PSUM") as pc, \
                 tc.tile_pool(name="pC_sb", bufs=4) as pcs:
             if b + 1 < B:
                # prefetch next batch's first x tiles behind C's outputs
                for dc in range(6):
                    xt = px.tile([128, LQ], bf16, tag="xt")
                    nc.sync.dma_start(
                        xt[:], xT[b + 1, dc * 128:(dc + 1) * 128, 0:LQ])
                    xt_pre[(b + 1, 0, dc)] = xt
             for lt in range(NLT):
                for hf in range(2):
                    wo_ps = pc.tile([128, D // 2], f32, tag="wo_ps")
                    for h in range(HPC):
                        for ec in range(2):
                            o = hf * 2 + ec
                            nc.tensor.matmul(
                                wo_ps[:, ec * LQ:(ec + 1) * LQ],
                                oTs_sb[h][lt // 4][:, (lt % 4) * 128:
                                                   (lt % 4 + 1) * 128],
                                wo_sb[:, h, o * LQ:(o + 1) * LQ],
                                start=(h == 0), stop=(h == HPC - 1))
                    ob = pcs.tile([128, D // 2], bf16, tag="ob")
                    if hf == 0:
                        nc.scalar.copy(ob[:], wo_ps[:])
                    else:
                        nc.vector.tensor_copy(ob[:], wo_ps[:])
                    nc.sync.dma_start(
                        part[b, lt * 128:(lt + 1) * 128,
                             hf * (D // 2):(hf + 1) * (D // 2)], ob[:])

    nc.compile()
    return nc


def _get_compiled(with_bias):
    if with_bias not in _compiled:
        _compiled[with_bias] = _build(with_bias)
    return _compiled[with_bias]


def _make_in_maps(query, Wq, bq, Wk, bk, Wv, bv, Wo, with_bias):
    import ml_dtypes
    bf = ml_dtypes.bfloat16

    xT = np.ascontiguousarray(query.transpose(0, 2, 1)).astype(bf)  # [B, D, L]
    cosT, sinT = _rope_tables_T()
    tri = _tri().astype(bf)

    def pack(wT, n):
        # [n*128, e] -> [128, n, e] so the SBUF load is per-partition contiguous
        e = wT.shape[1]
        return np.ascontiguousarray(
            wT.reshape(n, 128, e).transpose(1, 0, 2)).astype(bf)

    in_maps = []
    for c in range(NCORES):
        sl = slice(c * EPC, (c + 1) * EPC)
        m = {
            "xT": xT,
            "wqT": pack(Wq[sl].T, NDC),
            "wkT": pack(Wk[sl].T, NDC),
            "wvT": pack(Wv[sl].T, NDC),
            "woT": pack(Wo[:, sl].T, HPC),
            "cosT": cosT,
            "sinT": sinT,
            "tri": tri,
        }
        if with_bias:
            m["bq"] = np.ascontiguousarray(bq[sl][None, :]).astype(bf)
            m["bk"] = np.ascontiguousarray(bk[sl][None, :]).astype(bf)
            m["bv"] = np.ascontiguousarray(bv[sl][None, :]).astype(bf)
        in_maps.append(m)
    return in_maps


def kernel(query, Wq, bq, Wk, bk, Wv, bv, Wo, bo, _trace=False):
    from concourse.bass_utils import run_bass_kernel_spmd

    query = np.asarray(query, dtype=np.float32)
    Wq, Wk, Wv, Wo = (np.asarray(w, dtype=np.float32) for w in (Wq, Wk, Wv, Wo))
    bq_, bk_, bv_ = (np.asarray(x, dtype=np.float32) for x in (bq, bk, bv))
    bo = np.asarray(bo, dtype=np.float32)

    with_bias = bool(np.any(bq_) or np.any(bk_) or np.any(bv_))
    in_maps = _make_in_maps(query, Wq, bq_, Wk, bk_, Wv, bv_, Wo, with_bias)

    nc = _get_compiled(with_bias)
    res = run_bass_kernel_spmd(nc, in_maps, core_ids=list(range(NCORES)),
                               trace=_trace)
    out = np.zeros((B, L, D), dtype=np.float32)
    for r in res.results:
        out += r["part"].astype(np.float32)
    out += bo
    if _trace:
        kernel.last_exec_time_ns = res.exec_time_ns
        kernel.last_results = res
    return out.astype(np.float32)


# revision 28
# speedup vs baseline: 1.0455x; 1.0199x over previous
"""Multi-head attention (B=2, L=2048, D=2048, H=16, d=128, RoPE, causal)
on 8 Trainium2 NeuronCores, tensor-parallel over heads (2 heads/core).

Full-bf16 matmul path (halves LDWEIGHTS, keeps the HAM clock gate at
2.4 GHz via dense PE streams). Structure per batch:
  A) QKV projections accumulate in PSUM over 16 D-chunks; RoPE reads
     PSUM once via an ACT copy (GpSimd f32 muls + DVE crossed muls/add).
  B) scores [k,q] layout with per-512-q-chunk streams; diag tiles use
     restricted q-range + zero-memset tails so AV and the denominator
     (ones-stationary matmul into [1,LQ] PSUM) keep clean full-width
     accumulation groups; causal mask is one [128,128] tri-multiply on
     GpSimd per diag tile. Normalization (reciprocal -> ones-broadcast
     matmul -> DVE mul onto pre-scaled oT) is deferred two streams so
     the slow [1,LQ] reciprocal never stalls the in-order PE queue.
  C) output projection accumulates both heads in one PSUM group over
     half-D tiles (4-deep rotation), ACT/DVE alternate the bf16 copies.
State tensors (qr/kr/v/oTs/recT) are per-512-chunk tiles: the tile
framework tracks dependencies at tile granularity, so monolithic [HD,L]
tensors would serialize phase B behind all of A and C behind the last
normalization. Host sums the 8 bf16 partials + bo in f32.
"""

import sys

sys.path.insert(0, "/opt/trn_rl_repo")

import numpy as np

B = 2
L = 2048
D = 2048
H = 16
HD = 128          # head dim
NCORES = 8
HPC = H // NCORES  # heads per core = 2
EPC = HPC * HD     # output features per core = 256
LQ = 512           # q chunk width
NCH = L // LQ      # 4 q chunks
NDC = D // 128     # 16 contraction chunks
NLT = L // 128     # 16 l-tiles
SCALE = 1.0 / np.sqrt(HD)
ROPE_BASE = 10000.0

_compiled = {}


def _rope_tables_T():
    inv_freq = 1.0 / (ROPE_BASE ** (np.arange(0, HD, 2, dtype=np.float64) / HD))
    t = np.arange(L, dtype=np.float64)
    freqs = np.outer(t, inv_freq)                    # [L, 64]
    emb = np.concatenate([freqs, freqs], axis=-1)    # [L, 128]
    cosT = np.cos(emb).T.astype(np.float32)          # [128, L]
    sinT = np.sin(emb).T.astype(np.float32)
    sinT[0:64, :] *= -1.0                            # fold rotate-half sign
    return np.ascontiguousarray(cosT), np.ascontiguousarray(sinT)


def _tri():
    # tri[k, q] = 1.0 iff k <= q  (valid region inside a diagonal 128x128)
    i = np.arange(128)
    return (i[:, None] <= i[None, :]).astype(np.float32)


def _build(with_bias):
    import concourse.bass as bass
    import concourse.tile as tile
    from concourse import bacc, mybir

    f32 = mybir.dt.float32
    bf16 = mybir.dt.bfloat16
    nc = bacc.Bacc("TRN2", target_bir_lowering=False, debug=False,
                   enable_asserts=False, num_devices=NCORES)

    xT = nc.dram_tensor("xT", [B, D, L], bf16, kind="ExternalInput").ap()
    wqT = nc.dram_tensor("wqT", [128, NDC, EPC], bf16, kind="ExternalInput").ap()
    wkT = nc.dram_tensor("wkT", [128, NDC, EPC], bf16, kind="ExternalInput").ap()
    wvT = nc.dram_tensor("wvT", [128, NDC, EPC], bf16, kind="ExternalInput").ap()
    woT = nc.dram_tensor("woT", [128, HPC, D], bf16, kind="ExternalInput").ap()
    if with_bias:
        bq = nc.dram_tensor("bq", [1, EPC], bf16, kind="ExternalInput").ap()
        bk = nc.dram_tensor("bk", [1, EPC], bf16, kind="ExternalInput").ap()
        bv = nc.dram_tensor("bv", [1, EPC], bf16, kind="ExternalInput").ap()
    cosT = nc.dram_tensor("cosT", [HD, L], f32, kind="ExternalInput").ap()
    sinT = nc.dram_tensor("sinT", [HD, L], f32, kind="ExternalInput").ap()
    tri = nc.dram_tensor("tri", [128, 128], bf16, kind="ExternalInput").ap()
    part = nc.dram_tensor("part", [B, L, D], bf16, kind="ExternalOutput").ap()

    Exp = mybir.ActivationFunctionType.Exp

    with tile.TileContext(nc) as tc:
      with tc.tile_pool(name="consts", bufs=1) as consts, \
           tc.tile_pool(name="state", bufs=1) as state, \
           tc.tile_pool(name="pA_x", bufs=8) as px:
        # first x tiles for (b0,c0) go first, then weight chunks in dc
        # order so the dc-loop's dependencies land just-in-time
        wq_sb = [consts.tile([128, 4, EPC], bf16, tag=f"wq{g}", name=f"wq{g}")
                 for g in range(4)]
        wk_sb = [consts.tile([128, 4, EPC], bf16, tag=f"wk{g}", name=f"wk{g}")
                 for g in range(4)]
        wv_sb = [consts.tile([128, 4, EPC], bf16, tag=f"wv{g}", name=f"wv{g}")
                 for g in range(4)]
        # defined below; primed here so the g=0 weight chunks and the
        # first x tiles beat the remaining 4.5MB of weights onto the queues
        xt_pre = {}
        tri_sb = consts.tile([128, 128], bf16, tag="tri")
        for g in range(4):
            nc.sync.dma_start(wq_sb[g][:], wqT[:, g * 4:(g + 1) * 4, :])
            nc.sync.dma_start(wk_sb[g][:], wkT[:, g * 4:(g + 1) * 4, :])
            nc.sync.dma_start(wv_sb[g][:], wvT[:, g * 4:(g + 1) * 4, :])
            if g == 0:
                for dc in range(6):
                    xt = px.tile([128, LQ], bf16, tag="xt")
                    nc.sync.dma_start(
                        xt[:], xT[0, dc * 128:(dc + 1) * 128, 0:LQ])
                    xt_pre[(0, 0, dc)] = xt
        nc.sync.dma_start(tri_sb[:], tri)
        if with_bias:
            bq_sb = consts.tile([1, EPC], bf16, tag="bq")
            nc.sync.dma_start(bq_sb[:], bq)
            bk_sb = consts.tile([1, EPC], bf16, tag="bk")
            nc.sync.dma_start(bk_sb[:], bk)
            bv_sb = consts.tile([1, EPC], bf16, tag="bv")
            nc.sync.dma_start(bv_sb[:], bv)
            ones_row = consts.tile([1, LQ], bf16, tag="ones_row")
            nc.vector.memset(ones_row[:], 1.0)
        ones_col = consts.tile([128, 1], bf16, tag="ones_col")
        nc.vector.memset(ones_col[:], 1.0)
        ones_r128 = consts.tile([1, 128], bf16, tag="ones_r128")
        nc.vector.memset(ones_r128[:], 1.0)
        # cos/sin per chunk, DMA'd just-in-time from inside phase A so the
        # 4MB doesn't starve the xt stream; wo deferred to mid-A (phase C)
        cos_sb = [consts.tile([HD, LQ], f32, tag=f"cos{c}", name=f"cos{c}")
                  for c in range(NCH)]
        sin_sb = [consts.tile([HD, LQ], f32, tag=f"sin{c}", name=f"sin{c}")
                  for c in range(NCH)]
        wo_sb = consts.tile([128, HPC, D], bf16, tag="wo")

        # cross-batch xt prefetch: tiles whose DMA was already issued

        def xt_get(b, c, dc):
            key = (b, c, dc)
            if key in xt_pre:
                return xt_pre.pop(key)
            xt = px.tile([128, LQ], bf16, tag="xt")
            nc.sync.dma_start(
                xt[:], xT[b, dc * 128:(dc + 1) * 128, c * LQ:(c + 1) * LQ])
            return xt

        for b in range(B):
            # per-batch, per-chunk SBUF state (chunked to keep the tile
            # framework's dependency tracking fine-grained)
            qr_sb = [[state.tile([HD, LQ], bf16, tag=f"qr{h}_{c}", name=f"qr{h}_{c}")
                      for c in range(NCH)] for h in range(HPC)]
            kr_sb = [[state.tile([HD, LQ], bf16, tag=f"kr{h}_{c}", name=f"kr{h}_{c}")
                      for c in range(NCH)] for h in range(HPC)]
            v_sb = [state.tile([128, 4, EPC], bf16, tag=f"v_{c}", name=f"v_{c}")
                    for c in range(NCH)]
            oTs_sb = [[state.tile([HD, LQ], bf16, tag=f"oTs{h}_{c}", name=f"oTs{h}_{c}")
                       for c in range(NCH)] for h in range(HPC)]
            recT_sb = [[state.tile([1, LQ], bf16, tag=f"recT{h}_{c}", name=f"recT{h}_{c}")
                        for c in range(NCH)] for h in range(HPC)]

            # ---------------- Phase A: QKV projections + RoPE ----------------
            with tc.tile_pool(name="pA_ps", bufs=1, space="PSUM") as pA, \
                 tc.tile_pool(name="pA_scr", bufs=2) as psc, \
                 tc.tile_pool(name="pA_t", bufs=2) as prt:
             for c in range(NCH):
                if True:
                    q_ps = pA.tile([128, HPC, LQ], f32, tag="q_ps")
                    k_ps = pA.tile([128, HPC, LQ], f32, tag="k_ps")
                    v_ps = [pA.tile([128, EPC], f32, tag=f"v_ps{i}",
                                    name=f"v_ps{i}") for i in range(4)]
                    for dc in range(NDC):
                        xt = xt_get(b, c, dc)
                        if b == 0 and dc == 4:
                            nc.sync.dma_start(
                                cos_sb[c][:], cosT[:, c * LQ:(c + 1) * LQ])
                            nc.sync.dma_start(
                                sin_sb[c][:], sinT[:, c * LQ:(c + 1) * LQ])
                            if c == 2:
                                nc.sync.dma_start(wo_sb[:], woT)
                        st = dc == 0
                        sp = (not with_bias) and dc == NDC - 1
                        for h in range(HPC):
                            nc.tensor.matmul(
                                q_ps[:, h], wq_sb[dc // 4][:, dc % 4, h * HD:(h + 1) * HD],
                                xt[:], start=st, stop=sp)
                            nc.tensor.matmul(
                                k_ps[:, h], wk_sb[dc // 4][:, dc % 4, h * HD:(h + 1) * HD],
                                xt[:], start=st, stop=sp)
                        for lt in range(4):
                            nc.tensor.matmul(
                                v_ps[lt][:], xt[:, lt * 128:(lt + 1) * 128],
                                wv_sb[dc // 4][:, dc % 4, :], start=st, stop=sp)
                    if with_bias:
                        for h in range(HPC):
                            nc.tensor.matmul(
                                q_ps[:, h], bq_sb[:, h * HD:(h + 1) * HD],
                                ones_row[:], start=False, stop=True)
                            nc.tensor.matmul(
                                k_ps[:, h], bk_sb[:, h * HD:(h + 1) * HD],
                                ones_row[:], start=False, stop=True)
                        for lt in range(4):
                            nc.tensor.matmul(v_ps[lt][:], ones_r128[:],
                                             bv_sb[:], start=False, stop=True)

                    # free the v banks first: phase B's first PSUM tiles
                    # land on them
                    for lt in range(4):
                        nc.scalar.copy(v_sb[c][:, lt, :], v_ps[lt][:])
                    # RoPE: one ACT copy out of PSUM, aligned muls on GpSimd,
                    # crossed muls straight from PSUM on DVE, add+cast split
                    cs = cos_sb[c][:]
                    sn = sin_sb[c][:]
                    for src, dst in ((q_ps, qr_sb), (k_ps, kr_sb)):
                        scr = psc.tile([128, HPC, LQ], f32, tag="scr")
                        nc.scalar.copy(scr[:], src[:])
                        for h in range(HPC):
                            t1 = prt.tile([128, LQ], f32, tag="t1")
                            t2 = prt.tile([128, LQ], f32, tag="t2")
                            nc.gpsimd.tensor_mul(t1[:], scr[:, h], cs)
                            nc.vector.tensor_mul(t2[0:64, :], src[64:128, h],
                                                 sn[0:64, :])
                            nc.vector.tensor_mul(t2[64:128, :], src[0:64, h],
                                                 sn[64:128, :])
                            # add in f32 on Pool, cast on ACT: a dtype-
                            # converting tensor_tensor on DVE runs ~3x slower
                            nc.gpsimd.tensor_add(t1[:], t1[:], t2[:])
                            nc.scalar.copy(dst[h][c][:], t1[:])

            # ---------------- Phase B: attention ----------------
            with tc.tile_pool(name="pB_sc", bufs=3, space="PSUM") as pb, \
                 tc.tile_pool(name="pB_o", bufs=2, space="PSUM") as po, \
                 tc.tile_pool(name="pB_d", bufs=2, space="PSUM") as pd, \
                 tc.tile_pool(name="pB_r", bufs=1, space="PSUM") as pr, \
                 tc.tile_pool(name="pB_e", bufs=6) as pe, \
                 tc.tile_pool(name="pB_rs", bufs=4) as prs:
             norm_pend = []

             def norm_flush():
                # two-stream-deferred: the reciprocal is long done, so the
                # broadcast matmul doesn't stall the in-order PE queue
                h_, c_, oT_sb_, recc_ = norm_pend.pop(0)
                rec_ps = pr.tile([128, LQ], f32, tag="rec_ps")
                nc.tensor.matmul(rec_ps[:], ones_r128[:], recc_[:],
                                 start=True, stop=True)
                nc.vector.tensor_mul(oTs_sb[h_][c_][:], oT_sb_[:], rec_ps[:])

             for h in range(HPC):
                for c in range(NCH):
                    ntk = 4 * (c + 1)
                    out_ps = po.tile([HD, LQ], f32, tag="out_ps")
                    den_ps = pd.tile([1, LQ], f32, tag="den_ps")

                    def flush(t, et):
                        nc.tensor.matmul(
                            out_ps[:], v_sb[t // 4][:, t % 4, h * HD:(h + 1) * HD],
                            et[:], start=(t == 0), stop=(t == ntk - 1))
                        nc.tensor.matmul(
                            den_ps[:], ones_col[:],
                            et[:], start=(t == 0), stop=(t == ntk - 1))

                    pend = []
                    for t in range(ntk):
                        s = t - 4 * c
                        qlo = s * 128 if s > 0 else 0
                        tc_, tb = t // 4, t % 4
                        sc = pb.tile([128, LQ], f32, tag="sc")
                        nc.tensor.matmul(
                            sc[:, qlo:LQ],
                            kr_sb[h][tc_][:, tb * 128:(tb + 1) * 128],
                            qr_sb[h][c][:, qlo:LQ],
                            start=True, stop=True)
                        et = pe.tile([128, LQ], bf16, tag="et")
                        if qlo:
                            nc.gpsimd.memset(et[:, 0:qlo], 0.0)
                        nc.scalar.activation(et[:, qlo:LQ], sc[:, qlo:LQ],
                                             Exp, scale=SCALE)
                        if s >= 0:
                            nc.gpsimd.tensor_mul(et[:, qlo:qlo + 128],
                                                 et[:, qlo:qlo + 128], tri_sb[:])
                        pend.append((t, et))
                        if len(pend) > 2:
                            flush(*pend.pop(0))
                    while pend:
                        flush(*pend.pop(0))

                    # drain PSUM via fast ACT copies; reciprocal runs SBUF-side
                    den_sb = prs.tile([1, LQ], f32, tag="den_sb")
                    nc.scalar.copy(den_sb[:], den_ps[:])
                    oT_sb = prs.tile([HD, LQ], bf16, tag="oT_sb")
                    nc.scalar.copy(oT_sb[:], out_ps[:])
                    recc = recT_sb[h][c]
                    with nc.allow_low_precision(reason="softmax rec in bf16"):
                        nc.vector.reciprocal(recc[:], den_sb[:])
                    norm_pend.append((h, c, oT_sb, recc))
                    if len(norm_pend) > 2:
                        norm_flush()
             while norm_pend:
                norm_flush()

            # ---------------- Phase C: output projection ----------------
            with tc.tile_pool(name="pC_ps", bufs=4, space="PSUM") as pc, \
                 tc.tile_pool(name="pC_sb", bufs=4) as pcs:
             if b + 1 < B:
                # prefetch next batch's first x tiles behind C's outputs
                for dc in range(6):
                    xt = px.tile([128, LQ], bf16, tag="xt")
                    nc.sync.dma_start(
                        xt[:], xT[b + 1, dc * 128:(dc + 1) * 128, 0:LQ])
                    xt_pre[(b + 1, 0, dc)] = xt
             for lt in range(NLT):
                for hf in range(2):
                    wo_ps = pc.tile([128, D // 2], f32, tag="wo_ps")
                    for h in range(HPC):
                        for ec in range(2):
                            o = hf * 2 + ec
                            nc.tensor.matmul(
                                wo_ps[:, ec * LQ:(ec + 1) * LQ],
                                oTs_sb[h][lt // 4][:, (lt % 4) * 128:
                                                   (lt % 4 + 1) * 128],
                                wo_sb[:, h, o * LQ:(o + 1) * LQ],
                                start=(h == 0), stop=(h == HPC - 1))
                    ob = pcs.tile([128, D // 2], bf16, tag="ob")
                    if hf == 0:
                        nc.scalar.copy(ob[:], wo_ps[:])
                    else:
                        nc.vector.tensor_copy(ob[:], wo_ps[:])
                    nc.sync.dma_start(
                        part[b, lt * 128:(lt + 1) * 128,
                             hf * (D // 2):(hf + 1) * (D // 2)], ob[:])

    nc.compile()
    return nc


def _get_compiled(with_bias):
    if with_bias not in _compiled:
        _compiled[with_bias] = _build(with_bias)
    return _compiled[with_bias]


def _make_in_maps(query, Wq, bq, Wk, bk, Wv, bv, Wo, with_bias):
    import ml_dtypes
    bf = ml_dtypes.bfloat16

    xT = np.ascontiguousarray(query.transpose(0, 2, 1)).astype(bf)  # [B, D, L]
    cosT, sinT = _rope_tables_T()
    tri = _tri().astype(bf)

    def pack(wT, n):
        # [n*128, e] -> [128, n, e] so the SBUF load is per-partition contiguous
        e = wT.shape[1]
        return np.ascontiguousarray(
            wT.reshape(n, 128, e).transpose(1, 0, 2)).astype(bf)

    in_maps = []
    for c in range(NCORES):
        sl = slice(c * EPC, (c + 1) * EPC)
        m = {
            "xT": xT,
            "wqT": pack(Wq[sl].T, NDC),
            "wkT": pack(Wk[sl].T, NDC),
            "wvT": pack(Wv[sl].T, NDC),
            "woT": pack(Wo[:, sl].T, HPC),
            "cosT": cosT,
            "sinT": sinT,
            "tri": tri,
        }
        if with_bias:
            m["bq"] = np.ascontiguousarray(bq[sl][None, :]).astype(bf)
            m["bk"] = np.ascontiguousarray(bk[sl][None, :]).astype(bf)
            m["bv"] = np.ascontiguousarray(bv[sl][None, :]).astype(bf)
        in_maps.append(m)
    return in_maps


def kernel(query, Wq, bq, Wk, bk, Wv, bv, Wo, bo, _trace=False):
    from concourse.bass_utils import run_bass_kernel_spmd

    query = np.asarray(query, dtype=np.float32)
    Wq, Wk, Wv, Wo = (np.asarray(w, dtype=np.float32) for w in (Wq, Wk, Wv, Wo))
    bq_, bk_, bv_ = (np.asarray(x, dtype=np.float32) for x in (bq, bk, bv))
    bo = np.asarray(bo, dtype=np.float32)

    with_bias = bool(np.any(bq_) or np.any(bk_) or np.any(bv_))
    in_maps = _make_in_maps(query, Wq, bq_, Wk, bk_, Wv, bv_, Wo, with_bias)

    nc = _get_compiled(with_bias)
    res = run_bass_kernel_spmd(nc, in_maps, core_ids=list(range(NCORES)),
                               trace=_trace)
    out = np.zeros((B, L, D), dtype=np.float32)
    for r in res.results:
        out += r["part"].astype(np.float32)
    out += bo
    if _trace:
        kernel.last_exec_time_ns = res.exec_time_ns
        kernel.last_results = res
    return out.astype(np.float32)


# revision 29
# speedup vs baseline: 1.0518x; 1.0060x over previous
"""Multi-head attention (B=2, L=2048, D=2048, H=16, d=128, RoPE, causal)
on 8 Trainium2 NeuronCores, tensor-parallel over heads (2 heads/core).

Full-bf16 matmul path (halves LDWEIGHTS, keeps the HAM clock gate at
2.4 GHz via dense PE streams). Structure per batch:
  A) QKV projections accumulate in PSUM over 16 D-chunks; RoPE reads
     PSUM once via an ACT copy (GpSimd f32 muls + DVE crossed muls/add).
  B) scores [k,q] layout with per-512-q-chunk streams; diag tiles use
     restricted q-range + zero-memset tails so AV and the denominator
     (ones-stationary matmul into [1,LQ] PSUM) keep clean full-width
     accumulation groups; causal mask is one [128,128] tri-multiply on
     GpSimd per diag tile. Normalization (reciprocal -> ones-broadcast
     matmul -> DVE mul onto pre-scaled oT) is deferred two streams so
     the slow [1,LQ] reciprocal never stalls the in-order PE queue.
  C) output projection accumulates both heads in one PSUM group over
     half-D tiles (4-deep rotation), ACT/DVE alternate the bf16 copies.
State tensors (qr/kr/v/oTs/recT) are per-512-chunk tiles: the tile
framework tracks dependencies at tile granularity, so monolithic [HD,L]
tensors would serialize phase B behind all of A and C behind the last
normalization. Host sums the 8 bf16 partials + bo in f32.
"""

import sys

sys.path.insert(0, "/opt/trn_rl_repo")

import numpy as np

B = 2
L = 2048
D = 2048
H = 16
HD = 128          # head dim
NCORES = 8
HPC = H // NCORES  # heads per core = 2
EPC = HPC * HD     # output features per core = 256
LQ = 512           # q chunk width
NCH = L // LQ      # 4 q chunks
NDC = D // 128     # 16 contraction chunks
NLT = L // 128     # 16 l-tiles
SCALE = 1.0 / np.sqrt(HD)
ROPE_BASE = 10000.0

_compiled = {}


def _rope_tables_T():
    inv_freq = 1.0 / (ROPE_BASE ** (np.arange(0, HD, 2, dtype=np.float64) / HD))
    t = np.arange(L, dtype=np.float64)
    freqs = np.outer(t, inv_freq)                    # [L, 64]
    emb = np.concatenate([freqs, freqs], axis=-1)    # [L, 128]
    cosT = np.cos(emb).T.astype(np.float32)          # [128, L]
    sinT = np.sin(emb).T.astype(np.float32)
    sinT[0:64, :] *= -1.0                            # fold rotate-half sign
    return np.ascontiguousarray(cosT), np.ascontiguousarray(sinT)


def _tri():
    # tri[k, q] = 1.0 iff k <= q  (valid region inside a diagonal 128x128)
    i = np.arange(128)
    return (i[:, None] <= i[None, :]).astype(np.float32)


def _build(with_bias):
    import concourse.bass as bass
    import concourse.tile as tile
    from concourse import bacc, mybir

    f32 = mybir.dt.float32
    bf16 = mybir.dt.bfloat16
    nc = bacc.Bacc("TRN2", target_bir_lowering=False, debug=False,
                   enable_asserts=False, num_devices=NCORES)

    xT = nc.dram_tensor("xT", [B, D, L], bf16, kind="ExternalInput").ap()
    wqT = nc.dram_tensor("wqT", [128, NDC, EPC], bf16, kind="ExternalInput").ap()
    wkT = nc.dram_tensor("wkT", [128, NDC, EPC], bf16, kind="ExternalInput").ap()
    wvT = nc.dram_tensor("wvT", [128, NDC, EPC], bf16, kind="ExternalInput").ap()
    woT = nc.dram_tensor("woT", [128, HPC, D], bf16, kind="ExternalInput").ap()
    if with_bias:
        bq = nc.dram_tensor("bq", [1, EPC], bf16, kind="ExternalInput").ap()
        bk = nc.dram_tensor("bk", [1, EPC], bf16, kind="ExternalInput").ap()
        bv = nc.dram_tensor("bv", [1, EPC], bf16, kind="ExternalInput").ap()
    cosT = nc.dram_tensor("cosT", [HD, L], f32, kind="ExternalInput").ap()
    sinT = nc.dram_tensor("sinT", [HD, L], f32, kind="ExternalInput").ap()
    tri = nc.dram_tensor("tri", [128, 128], bf16, kind="ExternalInput").ap()
    part = nc.dram_tensor("part", [B, L, D], bf16, kind="ExternalOutput").ap()

    Exp = mybir.ActivationFunctionType.Exp

    with tile.TileContext(nc) as tc:
      with tc.tile_pool(name="consts", bufs=1) as consts, \
           tc.tile_pool(name="state", bufs=1) as state, \
           tc.tile_pool(name="pA_x", bufs=10) as px:
        # first x tiles for (b0,c0) go first, then weight chunks in dc
        # order so the dc-loop's dependencies land just-in-time
        wq_sb = [consts.tile([128, 4, EPC], bf16, tag=f"wq{g}", name=f"wq{g}")
                 for g in range(4)]
        wk_sb = [consts.tile([128, 4, EPC], bf16, tag=f"wk{g}", name=f"wk{g}")
                 for g in range(4)]
        wv_sb = [consts.tile([128, 4, EPC], bf16, tag=f"wv{g}", name=f"wv{g}")
                 for g in range(4)]
        # defined below; primed here so the g=0 weight chunks and the
        # first x tiles beat the remaining 4.5MB of weights onto the queues
        xt_pre = {}
        tri_sb = consts.tile([128, 128], bf16, tag="tri")
        for g in range(4):
            nc.sync.dma_start(wq_sb[g][:], wqT[:, g * 4:(g + 1) * 4, :])
            nc.sync.dma_start(wk_sb[g][:], wkT[:, g * 4:(g + 1) * 4, :])
            nc.sync.dma_start(wv_sb[g][:], wvT[:, g * 4:(g + 1) * 4, :])
            if g == 0:
                for dc in range(8):
                    xt = px.tile([128, LQ], bf16, tag="xt")
                    nc.sync.dma_start(
                        xt[:], xT[0, dc * 128:(dc + 1) * 128, 0:LQ])
                    xt_pre[(0, 0, dc)] = xt
        nc.sync.dma_start(tri_sb[:], tri)
        if with_bias:
            bq_sb = consts.tile([1, EPC], bf16, tag="bq")
            nc.sync.dma_start(bq_sb[:], bq)
            bk_sb = consts.tile([1, EPC], bf16, tag="bk")
            nc.sync.dma_start(bk_sb[:], bk)
            bv_sb = consts.tile([1, EPC], bf16, tag="bv")
            nc.sync.dma_start(bv_sb[:], bv)
            ones_row = consts.tile([1, LQ], bf16, tag="ones_row")
            nc.vector.memset(ones_row[:], 1.0)
        ones_col = consts.tile([128, 1], bf16, tag="ones_col")
        nc.vector.memset(ones_col[:], 1.0)
        ones_r128 = consts.tile([1, 128], bf16, tag="ones_r128")
        nc.vector.memset(ones_r128[:], 1.0)
        # cos/sin per chunk, DMA'd just-in-time from inside phase A so the
        # 4MB doesn't starve the xt stream; wo deferred to mid-A (phase C)
        cos_sb = [consts.tile([HD, LQ], f32, tag=f"cos{c}", name=f"cos{c}")
                  for c in range(NCH)]
        sin_sb = [consts.tile([HD, LQ], f32, tag=f"sin{c}", name=f"sin{c}")
                  for c in range(NCH)]
        wo_sb = consts.tile([128, HPC, D], bf16, tag="wo")

        # cross-batch xt prefetch: tiles whose DMA was already issued

        def xt_get(b, c, dc):
            key = (b, c, dc)
            if key in xt_pre:
                return xt_pre.pop(key)
            xt = px.tile([128, LQ], bf16, tag="xt")
            nc.sync.dma_start(
                xt[:], xT[b, dc * 128:(dc + 1) * 128, c * LQ:(c + 1) * LQ])
            return xt

        for b in range(B):
            # per-batch, per-chunk SBUF state (chunked to keep the tile
            # framework's dependency tracking fine-grained)
            qr_sb = [[state.tile([HD, LQ], bf16, tag=f"qr{h}_{c}", name=f"qr{h}_{c}")
                      for c in range(NCH)] for h in range(HPC)]
            kr_sb = [[state.tile([HD, LQ], bf16, tag=f"kr{h}_{c}", name=f"kr{h}_{c}")
                      for c in range(NCH)] for h in range(HPC)]
            v_sb = [state.tile([128, 4, EPC], bf16, tag=f"v_{c}", name=f"v_{c}")
                    for c in range(NCH)]
            oTs_sb = [[state.tile([HD, LQ], bf16, tag=f"oTs{h}_{c}", name=f"oTs{h}_{c}")
                       for c in range(NCH)] for h in range(HPC)]
            recT_sb = [[state.tile([1, LQ], bf16, tag=f"recT{h}_{c}", name=f"recT{h}_{c}")
                        for c in range(NCH)] for h in range(HPC)]

            # ---------------- Phase A: QKV projections + RoPE ----------------
            with tc.tile_pool(name="pA_ps", bufs=1, space="PSUM") as pA, \
                 tc.tile_pool(name="pA_scr", bufs=2) as psc, \
                 tc.tile_pool(name="pA_t", bufs=2) as prt:
             for c in range(NCH):
                if True:
                    q_ps = pA.tile([128, HPC, LQ], f32, tag="q_ps")
                    k_ps = pA.tile([128, HPC, LQ], f32, tag="k_ps")
                    v_ps = [pA.tile([128, EPC], f32, tag=f"v_ps{i}",
                                    name=f"v_ps{i}") for i in range(4)]
                    for dc in range(NDC):
                        xt = xt_get(b, c, dc)
                        if b == 0 and dc == 4:
                            nc.sync.dma_start(
                                cos_sb[c][:], cosT[:, c * LQ:(c + 1) * LQ])
                            nc.sync.dma_start(
                                sin_sb[c][:], sinT[:, c * LQ:(c + 1) * LQ])
                            if c == 2:
                                nc.sync.dma_start(wo_sb[:], woT)
                        st = dc == 0
                        sp = (not with_bias) and dc == NDC - 1
                        for h in range(HPC):
                            nc.tensor.matmul(
                                q_ps[:, h], wq_sb[dc // 4][:, dc % 4, h * HD:(h + 1) * HD],
                                xt[:], start=st, stop=sp)
                            nc.tensor.matmul(
                                k_ps[:, h], wk_sb[dc // 4][:, dc % 4, h * HD:(h + 1) * HD],
                                xt[:], start=st, stop=sp)
                        for lt in range(4):
                            nc.tensor.matmul(
                                v_ps[lt][:], xt[:, lt * 128:(lt + 1) * 128],
                                wv_sb[dc // 4][:, dc % 4, :], start=st, stop=sp)
                    if with_bias:
                        for h in range(HPC):
                            nc.tensor.matmul(
                                q_ps[:, h], bq_sb[:, h * HD:(h + 1) * HD],
                                ones_row[:], start=False, stop=True)
                            nc.tensor.matmul(
                                k_ps[:, h], bk_sb[:, h * HD:(h + 1) * HD],
                                ones_row[:], start=False, stop=True)
                        for lt in range(4):
                            nc.tensor.matmul(v_ps[lt][:], ones_r128[:],
                                             bv_sb[:], start=False, stop=True)

                    # free the v banks first: phase B's first PSUM tiles
                    # land on them
                    for lt in range(4):
                        nc.scalar.copy(v_sb[c][:, lt, :], v_ps[lt][:])
                    # RoPE: one ACT copy out of PSUM, aligned muls on GpSimd,
                    # crossed muls straight from PSUM on DVE, add+cast split
                    cs = cos_sb[c][:]
                    sn = sin_sb[c][:]
                    for src, dst in ((q_ps, qr_sb), (k_ps, kr_sb)):
                        scr = psc.tile([128, HPC, LQ], f32, tag="scr")
                        nc.scalar.copy(scr[:], src[:])
                        for h in range(HPC):
                            t1 = prt.tile([128, LQ], f32, tag="t1")
                            t2 = prt.tile([128, LQ], f32, tag="t2")
                            nc.gpsimd.tensor_mul(t1[:], scr[:, h], cs)
                            nc.vector.tensor_mul(t2[0:64, :], src[64:128, h],
                                                 sn[0:64, :])
                            nc.vector.tensor_mul(t2[64:128, :], src[0:64, h],
                                                 sn[64:128, :])
                            # add in f32 on Pool, cast on ACT: a dtype-
                            # converting tensor_tensor on DVE runs ~3x slower
                            nc.gpsimd.tensor_add(t1[:], t1[:], t2[:])
                            nc.scalar.copy(dst[h][c][:], t1[:])

            # ---------------- Phase B: attention ----------------
            with tc.tile_pool(name="pB_sc", bufs=3, space="PSUM") as pb, \
                 tc.tile_pool(name="pB_o", bufs=2, space="PSUM") as po, \
                 tc.tile_pool(name="pB_d", bufs=2, space="PSUM") as pd, \
                 tc.tile_pool(name="pB_r", bufs=1, space="PSUM") as pr, \
                 tc.tile_pool(name="pB_e", bufs=6) as pe, \
                 tc.tile_pool(name="pB_rs", bufs=4) as prs:
             norm_pend = []

             def norm_flush():
                # two-stream-deferred: the reciprocal is long done, so the
                # broadcast matmul doesn't stall the in-order PE queue
                h_, c_, oT_sb_, recc_ = norm_pend.pop(0)
                rec_ps = pr.tile([128, LQ], f32, tag="rec_ps")
                nc.tensor.matmul(rec_ps[:], ones_r128[:], recc_[:],
                                 start=True, stop=True)
                nc.vector.tensor_mul(oTs_sb[h_][c_][:], oT_sb_[:], rec_ps[:])

             for h in range(HPC):
                for c in range(NCH):
                    ntk = 4 * (c + 1)
                    out_ps = po.tile([HD, LQ], f32, tag="out_ps")
                    den_ps = pd.tile([1, LQ], f32, tag="den_ps")

                    def flush(t, et):
                        nc.tensor.matmul(
                            out_ps[:], v_sb[t // 4][:, t % 4, h * HD:(h + 1) * HD],
                            et[:], start=(t == 0), stop=(t == ntk - 1))
                        nc.tensor.matmul(
                            den_ps[:], ones_col[:],
                            et[:], start=(t == 0), stop=(t == ntk - 1))

                    pend = []
                    for t in range(ntk):
                        s = t - 4 * c
                        qlo = s * 128 if s > 0 else 0
                        tc_, tb = t // 4, t % 4
                        sc = pb.tile([128, LQ], f32, tag="sc")
                        nc.tensor.matmul(
                            sc[:, qlo:LQ],
                            kr_sb[h][tc_][:, tb * 128:(tb + 1) * 128],
                            qr_sb[h][c][:, qlo:LQ],
                            start=True, stop=True)
                        et = pe.tile([128, LQ], bf16, tag="et")
                        if qlo:
                            nc.gpsimd.memset(et[:, 0:qlo], 0.0)
                        nc.scalar.activation(et[:, qlo:LQ], sc[:, qlo:LQ],
                                             Exp, scale=SCALE)
                        if s >= 0:
                            nc.gpsimd.tensor_mul(et[:, qlo:qlo + 128],
                                                 et[:, qlo:qlo + 128], tri_sb[:])
                        pend.append((t, et))
                        if len(pend) > 3:
                            flush(*pend.pop(0))
                    while pend:
                        flush(*pend.pop(0))

                    # drain PSUM via fast ACT copies; reciprocal runs SBUF-side
                    den_sb = prs.tile([1, LQ], f32, tag="den_sb")
                    nc.scalar.copy(den_sb[:], den_ps[:])
                    oT_sb = prs.tile([HD, LQ], bf16, tag="oT_sb")
                    nc.scalar.copy(oT_sb[:], out_ps[:])
                    recc = recT_sb[h][c]
                    with nc.allow_low_precision(reason="softmax rec in bf16"):
                        nc.vector.reciprocal(recc[:], den_sb[:])
                    norm_pend.append((h, c, oT_sb, recc))
                    if len(norm_pend) > 2:
                        norm_flush()
             while norm_pend:
                norm_flush()

            # ---------------- Phase C: output projection ----------------
            with tc.tile_pool(name="pC_ps", bufs=4, space="PSUM") as pc, \
                 tc.tile_pool(name="pC_sb", bufs=4) as pcs:
             if b + 1 < B:
                # prefetch next batch's first x tiles behind C's outputs
                for dc in range(8):
                    xt = px.tile([128, LQ], bf16, tag="xt")
                    nc.sync.dma_start(
                        xt[:], xT[b + 1, dc * 128:(dc + 1) * 128, 0:LQ])
                    xt_pre[(b + 1, 0, dc)] = xt
             for lt in range(NLT):
                for hf in range(2):
                    wo_ps = pc.tile([128, D // 2], f32, tag="wo_ps")
                    for h in range(HPC):
                        for ec in range(2):
                            o = hf * 2 + ec
                            nc.tensor.matmul(
                                wo_ps[:, ec * LQ:(ec + 1) * LQ],
                                oTs_sb[h][lt // 4][:, (lt % 4) * 128:
                                                   (lt % 4 + 1) * 128],
                                wo_sb[:, h, o * LQ:(o + 1) * LQ],
                                start=(h == 0), stop=(h == HPC - 1))
                    ob = pcs.tile([128, D // 2], bf16, tag="ob")
                    if hf == 0:
                        nc.scalar.copy(ob[:], wo_ps[:])
                    else:
                        nc.vector.tensor_copy(ob[:], wo_ps[:])
                    nc.sync.dma_start(
                        part[b, lt * 128:(lt + 1) * 128,
                             hf * (D // 2):(hf + 1) * (D // 2)], ob[:])

    nc.compile()
    return nc


def _get_compiled(with_bias):
    if with_bias not in _compiled:
        _compiled[with_bias] = _build(with_bias)
    return _compiled[with_bias]


def _make_in_maps(query, Wq, bq, Wk, bk, Wv, bv, Wo, with_bias):
    import ml_dtypes
    bf = ml_dtypes.bfloat16

    xT = np.ascontiguousarray(query.transpose(0, 2, 1)).astype(bf)  # [B, D, L]
    cosT, sinT = _rope_tables_T()
    tri = _tri().astype(bf)

    def pack(wT, n):
        # [n*128, e] -> [128, n, e] so the SBUF load is per-partition contiguous
        e = wT.shape[1]
        return np.ascontiguousarray(
            wT.reshape(n, 128, e).transpose(1, 0, 2)).astype(bf)

    in_maps = []
    for c in range(NCORES):
        sl = slice(c * EPC, (c + 1) * EPC)
        m = {
            "xT": xT,
            "wqT": pack(Wq[sl].T, NDC),
            "wkT": pack(Wk[sl].T, NDC),
            "wvT": pack(Wv[sl].T, NDC),
            "woT": pack(Wo[:, sl].T, HPC),
            "cosT": cosT,
            "sinT": sinT,
            "tri": tri,
        }
        if with_bias:
            m["bq"] = np.ascontiguousarray(bq[sl][None, :]).astype(bf)
            m["bk"] = np.ascontiguousarray(bk[sl][None, :]).astype(bf)
            m["bv"] = np.ascontiguousarray(bv[sl][None, :]).astype(bf)
        in_maps.append(m)
    return in_maps


def kernel(query, Wq, bq, Wk, bk, Wv, bv, Wo, bo, _trace=False):
    from concourse.bass_utils import run_bass_kernel_spmd

    query = np.asarray(query, dtype=np.float32)
    Wq, Wk, Wv, Wo = (np.asarray(w, dtype=np.float32) for w in (Wq, Wk, Wv, Wo))
    bq_, bk_, bv_ = (np.asarray(x, dtype=np.float32) for x in (bq, bk, bv))
    bo = np.asarray(bo, dtype=np.float32)

    with_bias = bool(np.any(bq_) or np.any(bk_) or np.any(bv_))
    in_maps = _make_in_maps(query, Wq, bq_, Wk, bk_, Wv, bv_, Wo, with_bias)

    nc = _get_compiled(with_bias)
    res = run_bass_kernel_spmd(nc, in_maps, core_ids=list(range(NCORES)),
                               trace=_trace)
    out = np.zeros((B, L, D), dtype=np.float32)
    for r in res.results:
        out += r["part"].astype(np.float32)
    out += bo
    if _trace:
        kernel.last_exec_time_ns = res.exec_time_ns
        kernel.last_results = res
    return out.astype(np.float32)
